# revision 25
# baseline (speedup 1.0000x reference)
"""GAT (2-layer, 4-head) Trainium2 kernel over 8 NeuronCores.

Strategy:
  * Edges sorted by dst, dst-range partitioned across the 8 cores (each core
    owns N/8 node rows and fully computes their output -> no output
    all-reduce, softmax stats stay core-local).
  * Per layer: node GEMM is data-parallel over the owned node range, the
    node table h (pure 512B rows) is AllGathered to every core's HBM, then
    the edge phase gathers h[src] rows with dma_gather; s_src is computed
    on-device from the gathered rows (DVE mult+reduce against a_src), and
    the segment softmax + weighted scatter-add run as one-hot matmuls
    accumulated in PSUM (the wexp column yields the per-(dst,head) exp-sums
    in the same matmul).
  * int16 gather indices cap at 32767, so the node table is addressed as two
    halves (rows < 32768 and the rest) with per-(node-tile) A/B edge groups.
  * All host-prepared inputs ride in two blob parameters (bf16 + f32) to
    minimize per-iteration PJRT dispatch overhead.
"""

import sys

if "/opt/trn_rl_repo" not in sys.path:
    sys.path.insert(0, "/opt/trn_rl_repo")

import ml_dtypes
import numpy as np

import concourse.bacc as bacc
import concourse.bass as bass
import concourse.mybir as mybir
import concourse.tile as tile
from concourse.bass_utils import run_bass_kernel_spmd

BF16 = mybir.dt.bfloat16
F32 = mybir.dt.float32
I16 = mybir.dt.int16
I32 = mybir.dt.int32

NCORES = 8
P = 128

CFG = dict(
    N=50000,
    E=500000,
    F=256,      # feature width (in = hid = 256)
    H=4,
    DH=64,
    OUT=64,
    ROW=256,    # bf16 row length of node table (512B)
    VH=32768,   # int16 index split point
    G=2,        # node tiles per gather group
)


# --------------------------------------------------------------------------
# host-side preparation
# --------------------------------------------------------------------------

def _head_matrix(a):
    """[H, DH] -> block diagonal [F, H] so that s = h @ A."""
    H, DH = np.asarray(a).shape
    A = np.zeros((H * DH, H), np.float64)
    for h in range(H):
        A[h * DH:(h + 1) * DH, h] = np.asarray(a, np.float64)[h]
    return A


def _wfull(W, a_dst):
    """[W^T | W^T@Adst] as [F, 260] bf16."""
    W = np.asarray(W, np.float64)
    F = W.shape[1]
    Wt = W.T
    Bd = Wt @ _head_matrix(a_dst)
    out = np.zeros((F, 260), np.float64)
    out[:, :W.shape[0]] = Wt
    out[:, 256:260] = Bd
    return out.astype(ml_dtypes.bfloat16)


def _bn_consts(gamma, beta, mean, var, eps=1e-5):
    gamma = np.asarray(gamma, np.float64)
    beta = np.asarray(beta, np.float64)
    mean = np.asarray(mean, np.float64)
    var = np.asarray(var, np.float64)
    g = gamma / np.sqrt(var + eps)
    b = beta - mean * g
    F = gamma.shape[0]
    # [P, F//P]: col fc holds features fc*128 .. fc*128+127 on partitions
    return (
        np.ascontiguousarray(g.reshape(F // P, P).T.astype(np.float32)),
        np.ascontiguousarray(b.reshape(F // P, P).T.astype(np.float32)),
    )


def _wrap_idx(flat):
    """int16 position array -> dma_gather wrapped layout [128, len//16]."""
    n = len(flat)
    assert n % 16 == 0
    w = np.zeros((P, n // 16), np.int16)
    w[:16, :] = np.asarray(flat, np.int16).reshape(-1, 16).T
    w[16:, :] = np.tile(w[:16, :], (7, 1))
    return w


def prep_edges(cfg, edge_index):
    """Sort/partition edges; build per-core gather indices + dstrel tables."""
    N, G = cfg["N"], cfg["G"]
    NB = ((N + NCORES - 1) // NCORES + P - 1) // P * P  # nodes per core (padded)
    NT = NB // P                                        # node tiles per core
    VH = min(cfg["VH"], NB * NCORES // 2)
    cfg["VH"] = VH
    src = np.asarray(edge_index[0], np.int64)
    dst = np.asarray(edge_index[1], np.int64)

    core = dst // NB
    tilein = (dst % NB) // P
    half = (src >= VH).astype(np.int64)
    order = np.lexsort((src, half, tilein, core))
    sc, tc, hc = core[order], tilein[order], half[order]
    ss, ds = src[order], dst[order]
    key = (sc * NT + tc) * 2 + hc
    bounds = np.searchsorted(key, np.arange(NCORES * NT * 2 + 1))
    lists = {}
    maxa = maxb = 1
    for k in range(NCORES):
        for t in range(NT):
            for h in (0, 1):
                j = (k * NT + t) * 2 + h
                i0, i1 = bounds[j], bounds[j + 1]
                lists[(k, t, h)] = (ss[i0:i1], ds[i0:i1] % P)
                if h == 0:
                    maxa = max(maxa, i1 - i0)
                else:
                    maxb = max(maxb, i1 - i0)
    KA = int((maxa + P - 1) // P)
    KB = int((maxb + P - 1) // P)
    K = KA + KB

    groups = []
    t0 = 0
    while t0 < NT:
        groups.append((t0, min(G, NT - t0)))
        t0 += G

    per_core = []
    for k in range(NCORES):
        idxa_cols, idxb_cols, idxd_cols = [], [], []
        dstrel = np.full((P, NT * K), 128.0, np.float32)
        for (g0, gn) in groups:
            fa = np.zeros(gn * KA * P, np.int16)
            fb = np.zeros(gn * KB * P, np.int16)
            fd = np.zeros(gn * K * P, np.int16)
            for tl in range(gn):
                t = g0 + tl
                for h in (0, 1):
                    s_arr, r_arr = lists[(k, t, h)]
                    n = len(s_arr)
                    if h == 0:
                        fa[tl * KA * P: tl * KA * P + n] = s_arr.astype(np.int16)
                        col0 = g0 * K + tl * KA
                        slot0 = tl * KA
                    else:
                        fb[tl * KB * P: tl * KB * P + n] = (s_arr - VH).astype(np.int16)
                        col0 = g0 * K + gn * KA + tl * KB
                        slot0 = gn * KA + tl * KB
                    for i in range(n):
                        dstrel[i % P, col0 + i // P] = r_arr[i]
                    fd[slot0 * P: slot0 * P + n] = (t * P + r_arr).astype(np.int16)
            idxa_cols.append(_wrap_idx(fa))
            idxb_cols.append(_wrap_idx(fb))
            idxd_cols.append(_wrap_idx(fd))
        per_core.append(dict(
            idxa=np.concatenate(idxa_cols, axis=1),
            idxb=np.concatenate(idxb_cols, axis=1),
            idxd=np.concatenate(idxd_cols, axis=1),
            dstrel=dstrel.astype(ml_dtypes.bfloat16),
        ))
    return KA, KB, groups, per_core, NB, NT


# --------------------------------------------------------------------------
# device kernel
# --------------------------------------------------------------------------

def apx(base_ap, pairs, extra_offset=0):
    return bass.AP(base_ap.tensor, int(base_ap.offset + extra_offset),
                   [[int(p[0]), int(p[1])] for p in pairs])


def _layout16(cfg, KA, KB, groups, NB, NT):
    """Column offsets of each section inside the bf16 blob."""
    F, OUT = cfg["F"], cfg["OUT"]
    FC = F // P
    K = KA + KB
    SA = sum(gn * KA * 8 for _, gn in groups)
    SB = sum(gn * KB * 8 for _, gn in groups)
    SD = sum(gn * K * 8 for _, gn in groups)
    secs = [
        ("xt", FC * NB),
        ("wf1", FC * 260),
        ("wf2", FC * 260),
        ("wct", FC * OUT),
        ("dstrel", NT * K),
        ("iota", P),
        ("asrc1", F),
        ("asrc2", F),
        ("idxa", SA),
        ("idxb", SB),
        ("idxd", SD),
    ]
    off, out = 0, {}
    for name, ln in secs:
        out[name] = (off, ln)
        off += (ln + 1) // 2 * 2
    return out, off


LAYOUT32 = None


def _layout32(cfg):
    F, OUT = cfg["F"], cfg["OUT"]
    FC = F // P
    secs = [("gv1", FC), ("bv1", FC), ("gv2", FC), ("bv2", FC),
            ("bc", OUT), ("ident", P)]
    off, out = 0, {}
    for name, ln in secs:
        out[name] = (off, ln)
        off += ln
    return out, off


def build_kernel(cfg, KA, KB, groups, NB, NT):
    F, H, DH, OUT = cfg["F"], cfg["H"], cfg["DH"], cfg["OUT"]
    ROW, VH = cfg["ROW"], cfg["VH"]
    K = KA + KB
    FC = F // P
    NPAD = NB * NCORES
    AluOp = mybir.AluOpType
    Act = mybir.ActivationFunctionType

    nc = bacc.Bacc("TRN2", target_bir_lowering=False, debug=False,
                   num_devices=NCORES)
    ABL = cfg.get("ABL", 5)

    L16, C16 = _layout16(cfg, KA, KB, groups, NB, NT)
    L32, C32 = _layout32(cfg)

    # ---- I/O ----
    blob16 = nc.declare_dram_parameter("blob16", [P, C16], BF16, isOutput=False)
    blob32 = nc.declare_dram_parameter("blob32", [P, C32], F32, isOutput=False)
    out_ext = nc.declare_dram_parameter("out", [NB, OUT], F32, isOutput=True)

    b16 = blob16[:, :]
    b32 = blob32[:, :]

    def sec16(name, pairs, extra=0):
        off, _ = L16[name]
        return apx(b16, [b16.ap[0]] + [list(p) for p in pairs], off + extra)

    def sec32(name, pairs, extra=0):
        off, _ = L32[name]
        return apx(b32, [b32.ap[0]] + [list(p) for p in pairs], off + extra)

    haug_own = [nc.dram_tensor(f"haug_own{l}", [NB, ROW], BF16) for l in (0, 1)]
    sdst_tab = [nc.dram_tensor(f"sdst_tab{l}", [NB, P], BF16) for l in (0, 1)]
    haug_all = [nc.dram_tensor(f"haug_all{l}", [NCORES, NB, ROW], BF16,
                               addr_space="Local" if ABL == 3 else "Shared")
                for l in (0, 1)]

    with tile.TileContext(nc) as tc:
        with (
            tc.tile_pool(name="const", bufs=1) as cpool,
            tc.tile_pool(name="persist", bufs=1) as ppool,
            tc.tile_pool(name="work", bufs=3) as wpool,
            tc.tile_pool(name="smulp", bufs=2) as spool,
            tc.tile_pool(name="gath", bufs=2) as gpool,
            tc.tile_pool(name="psum", bufs=2, space="PSUM") as pspool,
            tc.tile_pool(name="psacc", bufs=2, space="PSUM") as accpool,
        ):
            # ---- constants ----
            ident = cpool.tile([P, P], F32)
            nc.sync.dma_start(out=ident[:, :], in_=sec32("ident", [[1, P]]))
            iota_bf = cpool.tile([P, P], BF16)
            nc.sync.dma_start(out=iota_bf[:, :], in_=sec16("iota", [[1, P]]))
            wf_sb = [cpool.tile([P, FC, 260], BF16, tag=f"wf{l}", name=f"wf{l}")
                     for l in range(2)]
            for l in range(2):
                nc.sync.dma_start(out=wf_sb[l][:, :, :],
                                  in_=sec16(f"wf{l + 1}", [[260, FC], [1, 260]]))
            wct_sb = cpool.tile([P, FC, OUT], BF16)
            nc.sync.dma_start(out=wct_sb[:, :, :],
                              in_=sec16("wct", [[OUT, FC], [1, OUT]]))
            gv_sb = [cpool.tile([P, FC], F32, tag=f"gv{l}", name=f"gv{l}") for l in range(2)]
            bv_sb = [cpool.tile([P, FC], F32, tag=f"bv{l}", name=f"bv{l}") for l in range(2)]
            for l in range(2):
                nc.sync.dma_start(out=gv_sb[l][:, :],
                                  in_=sec32(f"gv{l + 1}", [[1, FC]]))
                nc.sync.dma_start(out=bv_sb[l][:, :],
                                  in_=sec32(f"bv{l + 1}", [[1, FC]]))
            bc_sb = cpool.tile([P, OUT], F32)
            nc.sync.dma_start(out=bc_sb[:, :], in_=sec32("bc", [[1, OUT]]))
            ones1 = cpool.tile([1, P], BF16)
            nc.vector.memset(ones1[:, :], 1.0)
            bcb = cpool.tile([1, OUT], BF16)
            nc.scalar.copy(bcb[:, :], bc_sb[0:1, :])
            epsr = cpool.tile([1, 260], BF16)
            nc.vector.memset(epsr[:, 0:256], 0.0)
            nc.vector.memset(epsr[:, 256:260], 1e-9)
            dstrel_sb = cpool.tile([P, NT * K], BF16)
            nc.sync.dma_start(out=dstrel_sb[:, :],
                              in_=sec16("dstrel", [[1, NT * K]]))
            asrc_sb = [cpool.tile([P, F], BF16, tag=f"as{l}", name=f"as{l}")
                       for l in range(2)]
            for l in range(2):
                nc.sync.dma_start(out=asrc_sb[l][:, :],
                                  in_=sec16(f"asrc{l + 1}", [[1, F]]))

            # pre-zeroed staging tiles for the 4-wide sdst rows (only cols
            # 0:4 are rewritten per node tile)
            std_ring = [cpool.tile([P, P], BF16, tag=f"stdr{i}",
                                   name=f"stdr{i}") for i in range(3)]
            for srt in std_ring:
                nc.vector.memset(srt[:, :], 0.0)

            # ---- activations (transposed, bf16, SBUF resident) ----
            # two buffers ping-pong: layer0 reads [0] -> writes [1],
            # layer1 reads [1] -> writes [0], classifier reads [0].
            xt_sb = [ppool.tile([P, FC, NB], BF16, tag=f"xt{l}", name=f"xt{l}")
                     for l in range(2)]
            nc.sync.dma_start(out=xt_sb[0][:, :, :],
                              in_=sec16("xt", [[NB, FC], [1, NB]]))

            for rep_ in range(cfg.get("REPEAT", 1)):
                for layer in (0, 1):
                    xt = xt_sb[layer % 2]
                    xtn = xt_sb[(layer + 1) % 2]
                    wfl = wf_sb[layer]

                    # ---- node GEMM -> haug_own ----
                    for t in range(NT):
                        ps = pspool.tile([P, 260], F32, tag="gemm")
                        for kc in range(FC):
                            nc.tensor.matmul(
                                ps[:, :],
                                lhsT=xt[:, kc, t * P:(t + 1) * P],
                                rhs=wfl[:, kc, :],
                                start=(kc == 0), stop=(kc == FC - 1),
                            )
                        stg = wpool.tile([P, ROW], BF16, tag="gemmout")
                        nc.scalar.copy(stg[:, :], ps[:, 0:256])
                        nc.sync.dma_start(
                            out=haug_own[layer][t * P:(t + 1) * P, :],
                            in_=stg[:, :])
                        std = std_ring[t % len(std_ring)]
                        nc.scalar.copy(std[:, 0:4], ps[:, 256:260])
                        nc.sync.dma_start(
                            out=sdst_tab[layer][t * P:(t + 1) * P, :],
                            in_=std[:, :])

                    # ---- share node table ----
                    if ABL == 3:
                        for c in range(NCORES):
                            nc.sync.dma_start(out=haug_all[layer][c, :, :],
                                              in_=haug_own[layer][:, :])
                    elif ABL == 9:
                        nc.sync.dma_start(out=haug_all[layer][0, :, :],
                                          in_=haug_own[layer][:, :])
                    else:
                        reps = 2 if ABL == 10 else 1
                        for _ in range(reps):
                            nc.gpsimd.collective_compute(
                                "AllGather", AluOp.bypass,
                                replica_groups=[list(range(NCORES))],
                                ins=[haug_own[layer][:, :]],
                                outs=[haug_all[layer][:, :, :]],
                            )
                    hflat = haug_all[layer].rearrange("c n d -> (c n) d")
                    if ABL == 1:
                        nc.vector.memset(xtn[:, :, :], 0.1)
                        continue

                    # ---- edge phase ----
                    offa = offb = offd = 0
                    for (g0, gn) in groups:
                        npa, npb = gn * KA * P, gn * KB * P
                        nslot = gn * K
                        ia = wpool.tile([P, gn * KA * 8], I16, tag="ia")
                        ib = wpool.tile([P, gn * KB * 8], I16, tag="ib")
                        idt = wpool.tile([P, nslot * 8], I16, tag="idt")
                        nc.sync.dma_start(
                            out=ia[:, :],
                            in_=sec16("idxa", [[1, gn * KA * 8]], offa).bitcast(I16))
                        nc.sync.dma_start(
                            out=ib[:, :],
                            in_=sec16("idxb", [[1, gn * KB * 8]], offb).bitcast(I16))
                        nc.sync.dma_start(
                            out=idt[:, :],
                            in_=sec16("idxd", [[1, nslot * 8]], offd).bitcast(I16))
                        offa += gn * KA * 8
                        offb += gn * KB * 8
                        offd += nslot * 8

                        # slot layout per group: [gn*KA A-slots][gn*KB B-slots]
                        gat = gpool.tile([P, nslot, ROW], BF16, tag="gat")
                        sdg = gpool.tile([P, nslot, P], BF16, tag="sdg")
                        if ABL in (6, 7):
                            if ABL == 6:
                                nc.vector.memset(sdg[:, :, :], 0.05)
                                nc.vector.tensor_copy(sdg[:, 0:1, 0:8], idt[:, 0:8])
                                nc.gpsimd.dma_gather(
                                    out_ap=gat[:, 0:gn * KA, :], in_ap=hflat[0:VH, :],
                                    idxs_ap=ia[:, :], num_idxs=npa, num_idxs_reg=npa,
                                    elem_size=ROW, single_packet=False)
                                nc.gpsimd.dma_gather(
                                    out_ap=gat[:, gn * KA:nslot, :], in_ap=hflat[VH:NPAD, :],
                                    idxs_ap=ib[:, :], num_idxs=npb, num_idxs_reg=npb,
                                    elem_size=ROW, single_packet=False)
                            else:
                                nc.vector.memset(gat[:, :, :], 0.05)
                                nc.vector.tensor_copy(gat[:, 0:1, 0:8], ia[:, 0:8])
                                nc.vector.tensor_copy(gat[:, 1:2, 0:8], ib[:, 0:8])
                                nc.gpsimd.dma_gather(
                                    out_ap=sdg[:, :, :], in_ap=sdst_tab[layer][:, :],
                                    idxs_ap=idt[:, :], num_idxs=nslot * P,
                                    num_idxs_reg=nslot * P, elem_size=P,
                                    single_packet=False)
                        elif ABL == 2:
                            nc.vector.memset(gat[:, :, :], 0.05)
                            nc.vector.memset(sdg[:, :, :], 0.05)
                            nc.vector.tensor_copy(gat[:, 0:1, 0:8], ia[:, 0:8])
                            nc.vector.tensor_copy(gat[:, 1:2, 0:8], ib[:, 0:8])
                            nc.vector.tensor_copy(sdg[:, 0:1, 0:8], idt[:, 0:8])
                        else:
                            nc.gpsimd.dma_gather(
                                out_ap=gat[:, 0:gn * KA, :], in_ap=hflat[0:VH, :],
                                idxs_ap=ia[:, :], num_idxs=npa, num_idxs_reg=npa,
                                elem_size=ROW, single_packet=False)
                            nc.gpsimd.dma_gather(
                                out_ap=gat[:, gn * KA:nslot, :], in_ap=hflat[VH:NPAD, :],
                                idxs_ap=ib[:, :], num_idxs=npb, num_idxs_reg=npb,
                                elem_size=ROW, single_packet=False)
                            nc.gpsimd.dma_gather(
                                out_ap=sdg[:, :, :], in_ap=sdst_tab[layer][:, :],
                                idxs_ap=idt[:, :], num_idxs=nslot * P,
                                num_idxs_reg=nslot * P, elem_size=P,
                                single_packet=False)

                        # s_src on-device: reduce(gat * a_src) per head
                        smul = spool.tile([P, nslot, F], BF16, tag="smul")
                        asl = asrc_sb[layer][:, :]
                        nc.vector.tensor_tensor(
                            out=smul[:, :, :], in0=gat[:, :, :],
                            in1=apx(asl, [asl.ap[0], [0, nslot], [1, F]]),
                            op=AluOp.mult)
                        # pairwise-tree fold over DH (bf16, DVE 2x mode)
                        sm4 = smul[:, :, :].rearrange("p s (h d) -> p s h d", h=H)
                        w = DH // 2
                        while w >= 1:
                            nc.vector.tensor_tensor(
                                out=sm4[:, :, :, 0:w], in0=sm4[:, :, :, 0:w],
                                in1=sm4[:, :, :, w:2 * w], op=AluOp.add)
                            w //= 2
                        # e = lrelu(ssrc + sdst); w = exp(e)
                        ef = wpool.tile([P, nslot, H], F32, tag="ef")
                        nc.vector.tensor_tensor(
                            out=ef[:, :, :],
                            in0=sm4[:, :, :, 0:1].rearrange("p s h o -> p s (h o)"),
                            in1=sdg[:, :, 0:4], op=AluOp.add)
                        efl = wpool.tile([P, nslot, H], F32, tag="efl")
                        nc.scalar.activation(efl[:, :, :], ef[:, :, :],
                                             Act.Prelu, alpha=0.2)
                        wexp = wpool.tile([P, nslot, H], BF16, tag="wexp")
                        nc.scalar.activation(wexp[:, :, :], efl[:, :, :], Act.Exp)

                        # one-hot [128e, nslot, 128n]
                        oh = wpool.tile([P, nslot, P], BF16, tag="oh")
                        dr = dstrel_sb[:, g0 * K:g0 * K + nslot]
                        iota_ap = iota_bf[:, :]
                        nc.vector.tensor_tensor(
                            out=oh[:, :, :],
                            in0=apx(iota_ap, [iota_ap.ap[0], [0, nslot], [1, P]]),
                            in1=dr.to_broadcast([P, nslot, P]),
                            op=AluOp.is_equal)

                        # scale gathered rows by w; sums column = wexp itself
                        hts = wpool.tile([P, nslot, 260], BF16, tag="hts")
                        nc.vector.tensor_tensor(
                            out=hts[:, :, 0:256].rearrange("p s (h d) -> p s h d", h=H),
                            in0=gat[:, :, :].rearrange("p s (h d) -> p s h d", h=H),
                            in1=wexp[:, :, :].to_broadcast([P, nslot, H, DH]),
                            op=AluOp.mult)
                        nc.scalar.copy(hts[:, :, 256:260], wexp[:, :, :])

                        for tl in range(gn):
                            t = g0 + tl
                            acc = accpool.tile([P, 260], F32, tag="acc")
                            slots = ([tl * KA + s for s in range(KA)] +
                                     [gn * KA + tl * KB + s for s in range(KB)])
                            for j, sl in enumerate(slots):
                                nc.tensor.matmul(
                                    acc[:, :],
                                    lhsT=oh[:, sl, :],
                                    rhs=hts[:, sl, :],
                                    start=(j == 0), stop=False,
                                )
                            # T += 1e-9 (avoids a separate max(T, eps) pass)
                            nc.tensor.matmul(acc[:, :], lhsT=ones1[:, :],
                                             rhs=epsr[:, :], start=False,
                                             stop=True)
                            rec = wpool.tile([P, H], F32, tag="rec")
                            nc.vector.reciprocal(rec[:, :], acc[:, 256:260])
                            zsb = wpool.tile([P, F], F32, tag="zsb")
                            nc.vector.tensor_tensor(
                                out=zsb[:, :].rearrange("p (h d) -> p h d", h=H),
                                in0=acc[:, 0:256].rearrange("p (h d) -> p h d", h=H),
                                in1=rec[:, :].to_broadcast([P, H, DH]),
                                op=AluOp.mult)
                            # transpose + BN + ELU, both feature chunks in
                            # one [P, 2*P] block (scale/bias stay per-fc)
                            pst = pspool.tile([P, FC * P], F32, tag="ptr")
                            ybn = wpool.tile([P, FC * P], F32, tag="ybn")
                            ey = wpool.tile([P, FC * P], F32, tag="ey")
                            for fc in range(FC):
                                nc.tensor.transpose(
                                    pst[:, fc * P:(fc + 1) * P],
                                    zsb[:, fc * P:(fc + 1) * P], ident[:, :])
                                nc.scalar.activation(
                                    ybn[:, fc * P:(fc + 1) * P],
                                    pst[:, fc * P:(fc + 1) * P], Act.Identity,
                                    bias=bv_sb[layer][:, fc:fc + 1],
                                    scale=gv_sb[layer][:, fc:fc + 1])
                            nc.scalar.activation(ey[:, :], ybn[:, :], Act.Exp)
                            nc.vector.tensor_scalar(
                                out=ey[:, :], in0=ey[:, :], scalar1=1.0,
                                scalar2=0.0, op0=AluOp.subtract, op1=AluOp.min)
                            nc.scalar.activation(ybn[:, :], ybn[:, :], Act.Relu)
                            nc.vector.tensor_tensor(
                                out=xtn[:, :, t * P:(t + 1) * P],
                                in0=ey[:, :].rearrange("p (c n) -> p c n", c=FC),
                                in1=ybn[:, :].rearrange("p (c n) -> p c n", c=FC),
                                op=AluOp.add)

                # ---- classifier ----
                for t in range(NT):
                    ps = pspool.tile([P, OUT], F32, tag="cls")
                    for kc in range(FC):
                        nc.tensor.matmul(
                            ps[:, :],
                            lhsT=xt_sb[0][:, kc, t * P:(t + 1) * P],
                            rhs=wct_sb[:, kc, :],
                            start=(kc == 0), stop=False,
                        )
                    nc.tensor.matmul(ps[:, :], lhsT=ones1[:, :], rhs=bcb[:, :],
                                     start=False, stop=True)
                    ob = wpool.tile([P, OUT], F32, tag="ob")
                    nc.scalar.copy(ob[:, :], ps[:, :])
                    nc.sync.dma_start(out=out_ext[t * P:(t + 1) * P, :],
                                      in_=ob[:, :])

    nc.compile()
    return nc


# --------------------------------------------------------------------------
# entry point
# --------------------------------------------------------------------------

def kernel(x, edge_index, W1, a_src1, a_dst1, bn1_gamma, bn1_beta, bn1_mean,
           bn1_var, W2, a_src2, a_dst2, bn2_gamma, bn2_beta, bn2_mean, bn2_var,
           Wc, bc, _cfg=None, _run_kwargs=None, _bench=0):
    cfg = dict(CFG)
    if _cfg:
        cfg.update(_cfg)
    N, F, OUT = cfg["N"], cfg["F"], cfg["OUT"]
    FC = F // P

    KA, KB, groups, per_core, NB, NT = prep_edges(cfg, edge_index)
    nc = build_kernel(cfg, KA, KB, groups, NB, NT)
    L16, C16 = _layout16(cfg, KA, KB, groups, NB, NT)
    L32, C32 = _layout32(cfg)

    wfull1 = _wfull(W1, a_dst1)
    wfull2 = _wfull(W2, a_dst2)
    wct = np.ascontiguousarray(np.asarray(Wc, np.float64).T).astype(
        ml_dtypes.bfloat16)
    g1, b1 = _bn_consts(bn1_gamma, bn1_beta, bn1_mean, bn1_var)
    g2, b2 = _bn_consts(bn2_gamma, bn2_beta, bn2_mean, bn2_var)
    bc_rep = np.tile(np.asarray(bc, np.float32)[None, :], (P, 1))

    xpad = np.zeros((NB * NCORES, F), np.float32)
    xpad[:N] = np.asarray(x, np.float32)
    xt = np.ascontiguousarray(xpad.T).astype(ml_dtypes.bfloat16)  # [F, NPAD]

    def p_fc(arr_f_c):
        """[F(=FC*P), C] -> [P, FC*C] (feature-chunk-major per partition)."""
        Fdim, C = arr_f_c.shape
        return np.ascontiguousarray(
            arr_f_c.reshape(FC, P, C).transpose(1, 0, 2).reshape(P, FC * C))

    iota_row = np.tile(np.arange(P, dtype=np.float32)[None, :], (P, 1))
    asrc_rep1 = np.tile(np.asarray(a_src1, np.float32).reshape(1, F), (P, 1))
    asrc_rep2 = np.tile(np.asarray(a_src2, np.float32).reshape(1, F), (P, 1))

    bf = ml_dtypes.bfloat16

    def pack16(k):
        b = np.zeros((P, C16), bf)

        def put(name, arr):
            off, ln = L16[name]
            a = np.asarray(arr)
            assert a.shape == (P, ln), (name, a.shape, ln)
            b[:, off:off + ln] = a.astype(bf)

        put("xt", p_fc(xt[:, k * NB:(k + 1) * NB]))
        put("wf1", p_fc(np.asarray(wfull1, np.float32)))
        put("wf2", p_fc(np.asarray(wfull2, np.float32)))
        put("wct", p_fc(np.asarray(wct, np.float32)))
        put("dstrel", np.asarray(per_core[k]["dstrel"], np.float32))
        put("iota", iota_row)
        put("asrc1", asrc_rep1)
        put("asrc2", asrc_rep2)
        for nm in ("idxa", "idxb", "idxd"):
            off, ln = L16[nm]
            a = per_core[k][nm]
            assert a.shape == (P, ln), (nm, a.shape, ln)
            b[:, off:off + ln] = a.view(bf)
        return b

    b32 = np.zeros((P, C32), np.float32)
    for nm, arr in (("gv1", g1), ("bv1", b1), ("gv2", g2), ("bv2", b2),
                    ("bc", bc_rep), ("ident", np.eye(P, dtype=np.float32))):
        off, ln = L32[nm]
        b32[:, off:off + ln] = arr

    in_maps = [dict(blob16=pack16(k), blob32=b32) for k in range(NCORES)]

    res = run_bass_kernel_spmd(nc, in_maps, list(range(NCORES)),
                               **(_run_kwargs or {}))
    out = np.concatenate([res.results[k]["out"] for k in range(NCORES)], axis=0)
    out = out[:N].astype(np.float32)
    if _bench:
        ns = _bench_pjrt(nc, in_maps, _bench)
        return out, ns
    if _run_kwargs is not None:
        return out, res
    return out


def _bench_pjrt(nc, in_maps, iters):
    """Median per-iteration wall time (ns) of the NEFF execution via PJRT,
    device-resident inputs, back-to-back async dispatch."""
    import time
    import jax
    import jax.numpy as jnp
    from jax.sharding import Mesh, PartitionSpec
    from jax.experimental.shard_map import shard_map
    from concourse import bass2jax
    from concourse.bass2jax import _bass_exec_p, partition_id_tensor
    import concourse.mybir as mybir

    n_cores = len(in_maps)
    partition_name = nc.partition_id_tensor.name if nc.partition_id_tensor else None
    in_names, out_names, out_avals, zero_outs = [], [], [], []
    for alloc in nc.m.functions[0].allocations:
        if not isinstance(alloc, mybir.MemoryLocationSet):
            continue
        name = alloc.memorylocations[0].name
        if alloc.kind == "ExternalInput":
            if name != partition_name:
                in_names.append(name)
        elif alloc.kind == "ExternalOutput":
            shape = list(alloc.tensor_shape)
            dt = mybir.dt.np(alloc.dtype)
            out_avals.append(jax.core.ShapedArray(shape, dt))
            out_names.append(name)
            zero_outs.append(np.zeros(shape, dt))
    n_params = len(in_names)
    n_outs = len(out_names)
    in_names.extend(out_names)
    if partition_name is not None:
        in_names.append(partition_name)
    donate = tuple(range(n_params, n_params + n_outs))

    def _body(*args):
        operands = list(args)
        if partition_name is not None:
            operands.append(partition_id_tensor())
        return tuple(_bass_exec_p.bind(
            *operands, out_avals=tuple(out_avals), in_names=tuple(in_names),
            out_names=tuple(out_names), lowering_input_output_aliases=(),
            sim_require_finite=True, sim_require_nnan=True, nc=nc))

    devices = jax.devices()[:n_cores]
    mesh = Mesh(np.asarray(devices), ("core",))
    sharded = jax.jit(
        shard_map(_body, mesh=mesh,
                  in_specs=(PartitionSpec("core"),) * (n_params + n_outs),
                  out_specs=(PartitionSpec("core"),) * n_outs,
                  check_rep=False),
        donate_argnums=(), keep_unused=True)
    per_core = [[np.asarray(m[name]) for name in in_names[:n_params]]
                for m in in_maps]
    concat_in = [np.concatenate([per_core[c][i] for c in range(n_cores)], axis=0)
                 for i in range(n_params)]
    from jax.sharding import NamedSharding
    sh = NamedSharding(mesh, PartitionSpec("core"))
    dev_in = [jax.device_put(a, sh) for a in concat_in]
    zshapes = [(n_cores * z.shape[0], *z.shape[1:]) for z in zero_outs]
    zdtypes = [z.dtype for z in zero_outs]

    dev_zeros = [jax.device_put(np.zeros(s_, d_), sh)
                 for s_, d_ in zip(zshapes, zdtypes)]

    def one_iter():
        return sharded(*dev_in, *dev_zeros)

    jax.block_until_ready(one_iter())
    times = []
    for _ in range(5):
        t0 = time.perf_counter()
        outs = [one_iter() for _ in range(iters)]
        jax.block_until_ready(outs[-1])
        times.append((time.perf_counter() - t0) / iters * 1e9)
    return min(times)
